# revision 35
# baseline (speedup 1.0000x reference)
"""Trainium2 Bass kernel for nn_CrossAttention_79448305041860.

Dual cross-attention (q1, q2 vs shared kv) + concat + out-proj + LayerNorm,
B=4, E=256, N=64*64=4096 tokens.

Sharding: 8 cores = 4 batches x 2 query-token halves. Each core computes
K,V for its batch (replicated across the pair of cores sharing a batch) and
the full pipeline for its 2048-query-token slice. No cross-core comm.

Numerics: all matmul operands are bf16 (PE rate is identical to fp32r, but
bf16 enables FWL weight loads, 2x DVE element rate, and half the DMA/SBUF
traffic); PSUM accumulation and LN statistics stay fp32. Empirically this
lands at ~4.5e-3 rel err vs the 2e-2 gate.

Per-core structure:
  - K^T, Q^T computed e-major [e, tok] straight from channel-major inputs;
    V token-major. kv streams on the sync HWDGE queue, q-inputs + weights
    on the gpsimd SWDGE queue; the scalar queue carries NO DMA so the ACT
    FIFO is never head-blocked by DMA gates.
  - Attention processes k-tiles in PAIRS: scores for 2 k-tiles land in one
    2-bank PSUM tile, one ACT exp op covers [128, 2, 512], one DVE add
    accumulates the pair into the softmax-denominator accumulator.
  - PV matmuls lag the score matmuls by 2 pairs (software pipeline) so the
    PE never head-blocks on the exp latency.
  - Denominators: acc pair-fold + per-128q ones-matmul -> [q, 1], then
    reciprocal. Out-proj consumes unnormalized out^T halves as stationary
    operands; 1/denom applied as per-partition ACT scale at the PSUM drain.
  - LayerNorm rstd = 1/sqrt(var+eps) is computed on the DVE with the
    bit-trick rsqrt + 2 Newton steps, so the ACT engine never leaves the
    exp table set (no ACT_TABLE_LOAD thrash).
  - Out-proj + LN for token block qb are emitted right after the second
    attention pass over qb, hiding under the next block's attention and
    keeping the PE HAM-warm to the end. Final channel-major transpose is
    done by XBAR dma_start_transpose (bf16) on the sync queue -- no PE or
    DVE transpose work at all. Output is bf16, upcast to fp32 on host.
"""

import numpy as np
from contextlib import ExitStack

import ml_dtypes

import concourse.bass as bass
import concourse.mybir as mybir
import concourse.tile as tile
from concourse import bacc

FP32 = mybir.dt.float32
BF16 = mybir.dt.bfloat16
I32 = mybir.dt.int32
AF = mybir.ActivationFunctionType
ALU = mybir.AluOpType

P = 128
B = 4
E = 256            # embed dim
ET = E // P        # 2 e-tiles
CKV = 512          # kv channels
CT = CKV // P      # 4 c-tiles
CQ = 256           # q channels
CQT = CQ // P      # 2 c-tiles
N = 4096           # kv tokens per batch
NKT = N // P       # 32 k token-tiles
NPAIR = NKT // 2   # 16 k-tile pairs
NQ = 2048          # query tokens per core
QB = 512           # q block (psum bank width)
NQB = NQ // QB     # 4 q blocks
NT = NQ // P       # 16 token-tiles per core
TPB = QB // P      # 4 token-tiles per q block
SCALE = 1.0 / 16.0  # 1/sqrt(E)
LN_EPS = 1e-5
RSQRT_MAGIC = 0x5F3759DF
PVLAG = 2          # PV matmuls lag score matmuls by this many pairs


def _bcast_row(nc, dram_handle, sbuf_tile):
    """DMA-broadcast a [E] dram vector to all partitions of a [P, E] tile."""
    src_ap = dram_handle[:]
    bcast = bass.AP(
        tensor=src_ap.tensor,
        offset=src_ap.offset,
        ap=[[0, P], *src_ap.ap],
    )
    nc.gpsimd.dma_start(out=sbuf_tile[:], in_=bcast)


def build_nc():
    nc = bacc.Bacc()

    # weights / q-inputs arrive host-pre-arranged in the on-chip partition
    # layout ([p][o][...] contiguous) so DMA runs are 2-8KB, not 512B
    xq1_d = nc.dram_tensor("xq1", [P, CQT * NQ], BF16, kind="ExternalInput")
    xq2_d = nc.dram_tensor("xq2", [P, CQT * NQ], BF16, kind="ExternalInput")
    xkv_d = nc.dram_tensor("xkv", [CKV, N], BF16, kind="ExternalInput")
    wq1t_d = nc.dram_tensor("wq1t", [P, CQT * E], BF16, kind="ExternalInput")
    wq2t_d = nc.dram_tensor("wq2t", [P, CQT * E], BF16, kind="ExternalInput")
    wkt_d = nc.dram_tensor("wkt", [P, CT * E], BF16, kind="ExternalInput")
    wvt_d = nc.dram_tensor("wvt", [P, CT * E], BF16, kind="ExternalInput")
    wo1t_d = nc.dram_tensor("wo1t", [P, ET * E], BF16, kind="ExternalInput")
    wo2t_d = nc.dram_tensor("wo2t", [P, ET * E], BF16, kind="ExternalInput")
    bq1_d = nc.dram_tensor("bq1", [E], FP32, kind="ExternalInput")
    bq2_d = nc.dram_tensor("bq2", [E], FP32, kind="ExternalInput")
    bk_d = nc.dram_tensor("bk", [E], FP32, kind="ExternalInput")
    bv_d = nc.dram_tensor("bv", [E], FP32, kind="ExternalInput")
    bo_d = nc.dram_tensor("bo", [E], FP32, kind="ExternalInput")
    lnw_d = nc.dram_tensor("lnw", [E], FP32, kind="ExternalInput")
    lnb_d = nc.dram_tensor("lnb", [E], FP32, kind="ExternalInput")
    # token-major output; the host transposes to channel-major
    out_d = nc.dram_tensor("out", [NQ, E], FP32, kind="ExternalOutput")

    with tile.TileContext(nc) as tc, ExitStack() as ctx:
        const = ctx.enter_context(tc.tile_pool(name="const", bufs=1))
        wts = ctx.enter_context(tc.tile_pool(name="wts", bufs=1))
        bigin = ctx.enter_context(tc.tile_pool(name="bigin", bufs=1))
        keep = ctx.enter_context(tc.tile_pool(name="keep", bufs=1))
        flow = ctx.enter_context(tc.tile_pool(name="flow", bufs=1))
        # PSUM: 8 banks total = s(2x2) + o(2x1) + aux(2x1)
        ps_s = ctx.enter_context(tc.tile_pool(name="ps_s", bufs=2, space="PSUM"))
        ps_o = ctx.enter_context(tc.tile_pool(name="ps_o", bufs=2, space="PSUM"))
        ps_x = ctx.enter_context(tc.tile_pool(name="ps_x", bufs=2, space="PSUM"))

        # ---- weights / biases. wkt gates the very first matmuls, so it
        # rides the fast scalar HWDGE queue; everything else on gpsimd SWDGE.
        def _load_w(name, dram, ctiles, eng=None):
            t = wts.tile([P, ctiles, E], BF16, name=name)
            (eng or nc.gpsimd).dma_start(
                t[:], dram[:].rearrange("p (o e) -> p o e", o=ctiles)
            )
            return t

        wkt = _load_w("wkt", wkt_d, CT, eng=nc.scalar)
        bk = wts.tile([P, ET], FP32, name="bk")
        nc.scalar.dma_start(bk[:], bk_d[:].rearrange("(o p) -> p o", p=P))
        wvt = _load_w("wvt", wvt_d, CT, eng=nc.scalar)
        bv_b = wts.tile([P, E], FP32, name="bv_b")
        _bcast_row(nc, bv_d, bv_b)

        wq1t = _load_w("wq1t", wq1t_d, CQT)
        bq1 = wts.tile([P, ET], FP32, name="bq1")
        nc.gpsimd.dma_start(bq1[:], bq1_d[:].rearrange("(o p) -> p o", p=P))

        # PE warm-up: ~5us of throwaway matmuls during the DMA ramp flips the
        # HAM clock-gate to 8/8 before the first real projection matmuls.
        wrm = const.tile([P, QB], BF16, name="wrm")
        nc.vector.memset(wrm, 0.0)
        wps = ps_s.tile([P, 2, QB], FP32, name="wps", tag="s")
        NWARM = 8
        for wi in range(NWARM):
            nc.tensor.matmul(
                wps[:, 0, :], wrm[:, :P], wrm[:],
                start=(wi == 0), stop=(wi == NWARM - 1),
            )
        nc.vector.tensor_copy(wrm[:], wps[:, 0, :])
        # preload the exp table set while the PE is still streaming inputs
        nc.scalar.activation(wrm[:, 0:1], wrm[:, 0:1], AF.Exp, scale=1.0)

        QCH = 512
        wq2t = _load_w("wq2t", wq2t_d, CQT)
        bq2 = wts.tile([P, ET], FP32, name="bq2")
        nc.gpsimd.dma_start(bq2[:], bq2_d[:].rearrange("(o p) -> p o", p=P))
        wo1t = _load_w("wo1t", wo1t_d, ET)
        wo2t = _load_w("wo2t", wo2t_d, ET)
        bo_b = wts.tile([P, E], FP32, name="bo_b")
        _bcast_row(nc, bo_d, bo_b)
        lnw_b = wts.tile([P, E], FP32, name="lnw_b")
        _bcast_row(nc, lnw_d, lnw_b)
        lnb_b = wts.tile([P, E], FP32, name="lnb_b")
        _bcast_row(nc, lnb_d, lnb_b)

        # q inputs load whole (8KB contiguous runs per partition), LAST on
        # the gpsimd queue so they never contend with the early kv chunks
        xq1_sb = bigin.tile([P, CQT, NQ], BF16, name="xq1_sb")
        nc.gpsimd.dma_start(
            xq1_sb[:], xq1_d[:].rearrange("p (o n) -> p o n", o=CQT)
        )
        xq2_sb = bigin.tile([P, CQT, NQ], BF16, name="xq2_sb")
        nc.gpsimd.dma_start(
            xq2_sb[:], xq2_d[:].rearrange("p (o n) -> p o n", o=CQT)
        )

        # ---- constants ----
        ones = const.tile([P, 2], BF16, name="ones")
        nc.vector.memset(ones, 1.0)
        magic = const.tile([P, TPB], I32, name="magic")
        nc.vector.memset(magic, RSQRT_MAGIC)

        # ---- phase 0: K^T, V projections (kv on the sync HWDGE queue) ----
        ktm = keep.tile([P, ET, N], BF16, name="ktm")    # K^T e-major
        vtm = keep.tile([P, NKT, E], BF16, name="vtm")   # V token-major

        qt1 = keep.tile([P, CQT, NQ], BF16, name="qt1")  # Q1^T e-major
        qt2 = keep.tile([P, CQT, NQ], BF16, name="qt2")
        q_specs = [
            (xq_sb, wqt, bq, qt, ch)
            for (xq_sb, wqt, bq, qt) in (
                (xq1_sb, wq1t, bq1, qt1),
                (xq2_sb, wq2t, bq2, qt2),
            )
            for ch in range(NQ // QCH)
        ]

        # kv chunks: short ones first for a fast PE start, then long chunks
        # whose contiguous 2KB runs keep the DMA engines off the descriptor
        # bottleneck; alternating HWDGE queues (sync/scalar) double the rate
        KV_CHUNKS = [256, 256, 512, 1024, 1024, 1024]
        kv_off = 0
        for ci, kvch in enumerate(KV_CHUNKS):
            xkv_sb = bigin.tile([P, CT, 1024], BF16, name="xkv", tag="xkv", bufs=3)
            dma_eng = nc.sync if ci % 2 == 0 else nc.scalar
            dma_eng.dma_start(
                xkv_sb[:, :, :kvch],
                xkv_d[:].rearrange("(o p) n -> p o n", p=P)[
                    :, :, kv_off : kv_off + kvch
                ],
            )
            # K^T for these token-columns (ACT drains with per-partition bias)
            for t in range(ET):
                for cc in range(0, kvch, QB):
                    w = min(QB, kvch - cc)
                    ps = ps_o.tile([P, QB], FP32, name="kps", tag="o")
                    for j in range(CT):
                        nc.tensor.matmul(
                            ps[:, :w],
                            wkt[:, j, t * P : (t + 1) * P],
                            xkv_sb[:, j, cc : cc + w],
                            start=(j == 0),
                            stop=(j == CT - 1),
                        )
                    nc.scalar.activation(
                        ktm[:, t, kv_off + cc : kv_off + cc + w],
                        ps[:, :w],
                        AF.Identity,
                        bias=bk[:, t : t + 1],
                        scale=1.0,
                    )
            # V for these token-rows (DVE drains add bv)
            for v in range(kvch // P):
                kt_idx = (kv_off // P) + v
                ps = ps_x.tile([P, E], FP32, name="vps", tag="aux")
                for j in range(CT):
                    nc.tensor.matmul(
                        ps[:],
                        xkv_sb[:, j, v * P : (v + 1) * P],
                        wvt[:, j, :],
                        start=(j == 0),
                        stop=(j == CT - 1),
                    )
                nc.vector.tensor_tensor(vtm[:, kt_idx, :], ps[:], bv_b[:], ALU.add)
            kv_off += kvch

        def _qt_proj_chunk(i):
            """Project one q-input chunk into its Q^T slice."""
            xq_sb, wqt, bq, qt, ch = q_specs[i]
            csl = slice(ch * QCH, (ch + 1) * QCH)
            for t in range(ET):
                ps = ps_x.tile([P, QB], FP32, name="qps", tag="aux")
                for j in range(CQT):
                    nc.tensor.matmul(
                        ps[:],
                        wqt[:, j, t * P : (t + 1) * P],
                        xq_sb[:, j, csl],
                        start=(j == 0),
                        stop=(j == CQT - 1),
                    )
                nc.scalar.activation(
                    qt[:, t, ch * QCH : (ch + 1) * QCH],
                    ps[:],
                    AF.Identity,
                    bias=bq[:, t : t + 1],
                    scale=1.0,
                )

        # ---- phase 1 + interleaved phase 2 ----
        o1ut = keep.tile([P, ET, NQ], BF16, name="o1ut")  # unnormalized out1^T
        o2ut = keep.tile([P, ET, NQ], BF16, name="o2ut")
        r1 = keep.tile([P, NT], FP32, name="r1")          # 1/denom per token
        r2 = keep.tile([P, NT], FP32, name="r2")

        out_r = out_d[:].rearrange("(nt p) e -> p nt e", p=P)

        def _attn_span(si, qt, out_t, r_t, q_lo, q_w, next_chunk):
            """Attention k-sweep for queries [q_lo, q_lo+q_w)."""
            qsl = slice(q_lo, q_lo + q_w)
            o_ps = [
                ps_o.tile([P, QB], FP32, name=f"ops{t}", tag="o")
                for t in range(ET)
            ]
            acc = flow.tile([P, 2, QB], BF16, name="acc", tag="acc", bufs=2)

            def _emit_pv(pr, pt):
                for half in range(2):
                    k = 2 * pr + half
                    for t in range(ET):
                        nc.tensor.matmul(
                            o_ps[t][:, :q_w],
                            vtm[:, k, t * P : (t + 1) * P],
                            pt[:, half, :q_w],
                            start=(k == 0),
                            stop=(k == NKT - 1),
                        )

            pend = []
            for pr in range(NPAIR):
                # project the NEXT block's q chunk mid-way through this one,
                # so its qt slice is long done before that block starts
                if pr == NPAIR // 2 and next_chunk is not None:
                    _qt_proj_chunk(next_chunk)
                s_ps = ps_s.tile([P, 2, QB], FP32, name="sps", tag="s")
                for half in range(2):
                    k = 2 * pr + half
                    for t in range(ET):
                        nc.tensor.matmul(
                            s_ps[:, half, :q_w],
                            ktm[:, t, k * P : (k + 1) * P],
                            qt[:, t, qsl],
                            start=(t == 0),
                            stop=(t == ET - 1),
                        )
                pt = flow.tile([P, 2, QB], BF16, name="pt", tag="pt", bufs=4)
                nc.scalar.activation(
                    pt[:, :, :q_w], s_ps[:, :, :q_w], AF.Exp, scale=SCALE
                )
                pend.append((pr, pt))
                if len(pend) > PVLAG:
                    _emit_pv(*pend.pop(0))
                if pr == 0:
                    nc.vector.tensor_copy(acc[:, :, :q_w], pt[:, :, :q_w])
                else:
                    nc.vector.tensor_tensor(
                        acc[:, :, :q_w], acc[:, :, :q_w], pt[:, :, :q_w], ALU.add
                    )
            for args in pend:
                _emit_pv(*args)

            for t in range(ET):
                nc.vector.tensor_copy(out_t[:, t, qsl], o_ps[t][:, :q_w])
            # denominators: accumulating ones-matmuls over both acc halves
            d_ps = ps_x.tile([P, TPB, 2], FP32, name="dps", tag="aux")
            nsub = q_w // P
            for i in range(nsub):
                for h in range(2):
                    nc.tensor.matmul(
                        d_ps[:, i, :],
                        acc[:, h, i * P : (i + 1) * P],
                        ones[:],
                        start=(h == 0),
                        stop=(h == 1),
                    )
            nc.vector.reciprocal(
                r_t[:, q_lo // P : q_lo // P + nsub], d_ps[:, :nsub, 0]
            )

        def _phase2a(tiles):
            """Out-proj + softmax-normalize + LayerNorm for given token-tiles."""
            ntl = len(tiles)
            mv = flow.tile([P, TPB, 2], FP32, name="mv", tag="mv", bufs=2)
            ys_list = []
            for i, nt in enumerate(tiles):
                nsl = slice(nt * P, (nt + 1) * P)
                y_ps = ps_x.tile([P, 2, E], FP32, name="yps", tag="aux")
                for h, (out_t, wot) in enumerate(((o1ut, wo1t), (o2ut, wo2t))):
                    for j in range(ET):
                        nc.tensor.matmul(
                            y_ps[:, h, :],
                            out_t[:, j, nsl],
                            wot[:, j, :],
                            start=(j == 0),
                            stop=(j == ET - 1),
                        )
                yb = flow.tile([P, 2, E], FP32, name="yb", tag="yb", bufs=2)
                for h, r_t in enumerate((r1, r2)):
                    nc.scalar.activation(
                        yb[:, h, :], y_ps[:, h, :], AF.Identity,
                        scale=r_t[:, nt : nt + 1],
                    )
                ys = flow.tile([P, E], FP32, name="ys", tag="ys", bufs=2 * TPB)
                nc.vector.tensor_tensor(ys[:], yb[:, 0, :], yb[:, 1, :], ALU.add)
                nc.vector.tensor_tensor(ys[:], ys[:], bo_b[:], ALU.add)
                st6 = flow.tile([P, 6], FP32, name="st6", tag="st6", bufs=2)
                nc.vector.bn_stats(out=st6[:], in_=ys[:])
                nc.vector.bn_aggr(out=mv[:, i, :], in_=st6[:])
                ys_list.append(ys)
            # rstd = 1/sqrt(var+eps) on DVE (magic rsqrt + 2 Newton steps)
            rs = flow.tile([P, TPB], FP32, name="rs", tag="rs", bufs=2)
            t4 = flow.tile([P, TPB], FP32, name="t4", tag="t4", bufs=2)
            x4 = flow.tile([P, TPB], FP32, name="x4", tag="x4", bufs=2)
            nc.vector.tensor_scalar(
                x4[:, :ntl], mv[:, :ntl, 1], LN_EPS, None, op0=ALU.add
            )
            nc.vector.tensor_scalar(
                rs[:, :ntl].bitcast(I32), x4[:, :ntl].bitcast(I32), 1, None,
                op0=ALU.logical_shift_right,
            )
            nc.vector.tensor_tensor(
                rs[:, :ntl].bitcast(I32), magic[:, :ntl],
                rs[:, :ntl].bitcast(I32), ALU.subtract,
            )
            # one Newton step (max rel err ~1.8e-3 on rstd; well inside budget)
            for _ in range(1):
                nc.vector.tensor_tensor(t4[:, :ntl], x4[:, :ntl], rs[:, :ntl], ALU.mult)
                nc.vector.tensor_tensor(t4[:, :ntl], t4[:, :ntl], rs[:, :ntl], ALU.mult)
                nc.vector.tensor_scalar(
                    t4[:, :ntl], t4[:, :ntl], -0.5, 1.5, op0=ALU.mult, op1=ALU.add
                )
                nc.vector.tensor_tensor(rs[:, :ntl], rs[:, :ntl], t4[:, :ntl], ALU.mult)
            # normalize + affine, then store token-major (host transposes)
            for i, nt in enumerate(tiles):
                ys = ys_list[i]
                yf = flow.tile([P, E], FP32, name="yf", tag="yf", bufs=2 * TPB)
                nc.vector.tensor_scalar(
                    yf[:], ys[:], mv[:, i, 0:1], rs[:, i : i + 1],
                    op0=ALU.subtract, op1=ALU.mult,
                )
                nc.vector.tensor_tensor(yf[:], yf[:], lnw_b[:], ALU.mult)
                nc.vector.tensor_tensor(yf[:], yf[:], lnb_b[:], ALU.add)
                nc.sync.dma_start(out_r[:, nt, :], yf[:])

        _qt_proj_chunk(0)
        for qb in range(NQB):                      # set 1 (q1): attention only
            _attn_span(0, qt1, o1ut, r1, qb * QB, QB, qb + 1)
        for qb in range(NQB - 1):                  # set 2 (q2): attn + phase 2
            nxt = NQB + qb + 1 if NQB + qb + 1 < len(q_specs) else None
            _attn_span(1, qt2, o2ut, r2, qb * QB, QB, nxt)
            _phase2a([qb * TPB + i for i in range(TPB)])
        # final block split into two query halves so only ~2 token-tiles of
        # LN/store work remain exposed after the last matmul
        HB = QB // 2
        q0 = (NQB - 1) * QB
        _attn_span(1, qt2, o2ut, r2, q0, HB, None)
        _phase2a([q0 // P, q0 // P + 1])
        _attn_span(1, qt2, o2ut, r2, q0 + HB, HB, None)
        _phase2a([(q0 + HB) // P, (q0 + HB) // P + 1])

    nc.compile()
    return nc


_CACHE = {}


def _get_nc():
    if "nc" not in _CACHE:
        _CACHE["nc"] = build_nc()
    return _CACHE["nc"]


def make_in_maps(q1, q2, kv, wq1, bq1, wq2, bq2, wk, bk, wv, bv, wo, bo, ln_w, ln_b):
    bf = lambda a: np.ascontiguousarray(
        np.asarray(a, dtype=np.float32).astype(ml_dtypes.bfloat16)
    )
    f32 = lambda a: np.ascontiguousarray(np.asarray(a, dtype=np.float32))

    def sharded(wt):
        # [C, E] -> [P, (C//P)*E] in the on-chip [p][o][e] layout
        c, e = wt.shape
        return bf(wt.reshape(c // P, P, e).transpose(1, 0, 2).reshape(P, -1))

    q1, q2, kv = np.asarray(q1), np.asarray(q2), np.asarray(kv)
    base = {
        "wq1t": sharded(np.asarray(wq1).T),
        "wq2t": sharded(np.asarray(wq2).T),
        "wkt": sharded(np.asarray(wk).T),
        "wvt": sharded(np.asarray(wv).T),
        "wo1t": sharded(np.asarray(wo)[:, :E].T),
        "wo2t": sharded(np.asarray(wo)[:, E:].T),
        "bq1": f32(bq1),
        "bq2": f32(bq2),
        "bk": f32(bk),
        "bv": f32(bv),
        "bo": f32(bo),
        "lnw": f32(ln_w),
        "lnb": f32(ln_b),
    }
    kv_flat = [bf(kv[b].reshape(CKV, N)) for b in range(B)]
    in_maps = []
    for c in range(8):
        b, h = divmod(c, 2)
        m = dict(base)
        m["xq1"] = sharded(q1[b, :, h * 32 : (h + 1) * 32, :].reshape(CQ, NQ))
        m["xq2"] = sharded(q2[b, :, h * 32 : (h + 1) * 32, :].reshape(CQ, NQ))
        m["xkv"] = kv_flat[b]
        in_maps.append(m)
    return in_maps


def assemble_output(results):
    out = np.empty((B, E, 64, 64), dtype=np.float32)
    for c in range(8):
        b, h = divmod(c, 2)
        y = np.asarray(results[c]["out"], dtype=np.float32)  # [NQ, E]
        out[b, :, h * 32 : (h + 1) * 32, :] = y.T.reshape(E, 32, 64)
    return out


def kernel(**inputs):
    from concourse.bass_utils import run_bass_kernel_spmd

    nc = _get_nc()
    in_maps = make_in_maps(**inputs)
    res = run_bass_kernel_spmd(nc, in_maps, list(range(8)))
    return assemble_output(res.results)


if __name__ == "__main__":
    nc = build_nc()
    print("built ok")


# revision 40
# speedup vs baseline: 1.0215x; 1.0215x over previous
"""Trainium2 Bass kernel for nn_CrossAttention_79448305041860.

Dual cross-attention (q1, q2 vs shared kv) + concat + out-proj + LayerNorm,
B=4, E=256, N=64*64=4096 tokens.

Sharding: 8 cores = 4 batches x 2 query-token halves. Each core computes
K,V for its batch (replicated across the pair of cores sharing a batch) and
the full pipeline for its 2048-query-token slice. No cross-core comm.

Numerics: all matmul operands are bf16 (PE rate is identical to fp32r, but
bf16 enables FWL weight loads, 2x DVE element rate, and half the DMA/SBUF
traffic); PSUM accumulation and LN statistics stay fp32. Empirically this
lands at ~4.5e-3 rel err vs the 2e-2 gate.

Per-core structure:
  - K^T, Q^T computed e-major [e, tok] straight from channel-major inputs;
    V token-major. kv streams on the sync HWDGE queue, q-inputs + weights
    on the gpsimd SWDGE queue; the scalar queue carries NO DMA so the ACT
    FIFO is never head-blocked by DMA gates.
  - Attention processes k-tiles in PAIRS: scores for 2 k-tiles land in one
    2-bank PSUM tile, one ACT exp op covers [128, 2, 512], one DVE add
    accumulates the pair into the softmax-denominator accumulator.
  - PV matmuls lag the score matmuls by 2 pairs (software pipeline) so the
    PE never head-blocks on the exp latency.
  - Denominators: acc pair-fold + per-128q ones-matmul -> [q, 1], then
    reciprocal. Out-proj consumes unnormalized out^T halves as stationary
    operands; 1/denom applied as per-partition ACT scale at the PSUM drain.
  - LayerNorm rstd = 1/sqrt(var+eps) is computed on the DVE with the
    bit-trick rsqrt + 2 Newton steps, so the ACT engine never leaves the
    exp table set (no ACT_TABLE_LOAD thrash).
  - Out-proj + LN for token block qb are emitted right after the second
    attention pass over qb, hiding under the next block's attention and
    keeping the PE HAM-warm to the end. Final channel-major transpose is
    done by XBAR dma_start_transpose (bf16) on the sync queue -- no PE or
    DVE transpose work at all. Output is bf16, upcast to fp32 on host.
"""

import numpy as np
from contextlib import ExitStack

import ml_dtypes

import concourse.bass as bass
import concourse.mybir as mybir
import concourse.tile as tile
from concourse import bacc

FP32 = mybir.dt.float32
BF16 = mybir.dt.bfloat16
I32 = mybir.dt.int32
AF = mybir.ActivationFunctionType
ALU = mybir.AluOpType

P = 128
B = 4
E = 256            # embed dim
ET = E // P        # 2 e-tiles
CKV = 512          # kv channels
CT = CKV // P      # 4 c-tiles
CQ = 256           # q channels
CQT = CQ // P      # 2 c-tiles
N = 4096           # kv tokens per batch
NKT = N // P       # 32 k token-tiles
NPAIR = NKT // 2   # 16 k-tile pairs
NQ = 2048          # query tokens per core
QB = 512           # q block (psum bank width)
NQB = NQ // QB     # 4 q blocks
NT = NQ // P       # 16 token-tiles per core
TPB = QB // P      # 4 token-tiles per q block
SCALE = 1.0 / 16.0  # 1/sqrt(E)
LN_EPS = 1e-5
RSQRT_MAGIC = 0x5F3759DF
PVLAG = 2          # PV matmuls lag score matmuls by this many pairs


def _bcast_row(nc, dram_handle, sbuf_tile):
    """DMA-broadcast a [E] dram vector to all partitions of a [P, E] tile."""
    src_ap = dram_handle[:]
    bcast = bass.AP(
        tensor=src_ap.tensor,
        offset=src_ap.offset,
        ap=[[0, P], *src_ap.ap],
    )
    nc.gpsimd.dma_start(out=sbuf_tile[:], in_=bcast)


def build_nc():
    nc = bacc.Bacc()

    # weights / q-inputs arrive host-pre-arranged in the on-chip partition
    # layout ([p][o][...] contiguous) so DMA runs are 2-8KB, not 512B
    xq1_d = nc.dram_tensor("xq1", [P, CQT * NQ], BF16, kind="ExternalInput")
    xq2_d = nc.dram_tensor("xq2", [P, CQT * NQ], BF16, kind="ExternalInput")
    xkv_d = nc.dram_tensor("xkv", [CKV, N], BF16, kind="ExternalInput")
    wq1t_d = nc.dram_tensor("wq1t", [P, CQT * E], BF16, kind="ExternalInput")
    wq2t_d = nc.dram_tensor("wq2t", [P, CQT * E], BF16, kind="ExternalInput")
    wkt_d = nc.dram_tensor("wkt", [P, CT * E], BF16, kind="ExternalInput")
    wvt_d = nc.dram_tensor("wvt", [P, CT * E], BF16, kind="ExternalInput")
    wo1t_d = nc.dram_tensor("wo1t", [P, ET * E], BF16, kind="ExternalInput")
    wo2t_d = nc.dram_tensor("wo2t", [P, ET * E], BF16, kind="ExternalInput")
    bq1_d = nc.dram_tensor("bq1", [E], FP32, kind="ExternalInput")
    bq2_d = nc.dram_tensor("bq2", [E], FP32, kind="ExternalInput")
    bk_d = nc.dram_tensor("bk", [E], FP32, kind="ExternalInput")
    bv_d = nc.dram_tensor("bv", [E], FP32, kind="ExternalInput")
    bo_d = nc.dram_tensor("bo", [E], FP32, kind="ExternalInput")
    lnw_d = nc.dram_tensor("lnw", [E], FP32, kind="ExternalInput")
    lnb_d = nc.dram_tensor("lnb", [E], FP32, kind="ExternalInput")
    # token-major output; the host transposes to channel-major
    out_d = nc.dram_tensor("out", [NQ, E], FP32, kind="ExternalOutput")

    with tile.TileContext(nc) as tc, ExitStack() as ctx:
        const = ctx.enter_context(tc.tile_pool(name="const", bufs=1))
        wts = ctx.enter_context(tc.tile_pool(name="wts", bufs=1))
        bigin = ctx.enter_context(tc.tile_pool(name="bigin", bufs=1))
        keep = ctx.enter_context(tc.tile_pool(name="keep", bufs=1))
        flow = ctx.enter_context(tc.tile_pool(name="flow", bufs=1))
        # PSUM: 8 banks total = s(2x2) + o(2x1) + aux(2x1)
        ps_s = ctx.enter_context(tc.tile_pool(name="ps_s", bufs=2, space="PSUM"))
        ps_o = ctx.enter_context(tc.tile_pool(name="ps_o", bufs=2, space="PSUM"))
        ps_x = ctx.enter_context(tc.tile_pool(name="ps_x", bufs=2, space="PSUM"))

        # ---- weights / biases. wkt gates the very first matmuls, so it
        # rides the fast scalar HWDGE queue; everything else on gpsimd SWDGE.
        def _load_w(name, dram, ctiles, eng=None):
            t = wts.tile([P, ctiles, E], BF16, name=name)
            (eng or nc.gpsimd).dma_start(
                t[:], dram[:].rearrange("p (o e) -> p o e", o=ctiles)
            )
            return t

        wkt = _load_w("wkt", wkt_d, CT, eng=nc.scalar)
        bk = wts.tile([P, ET], FP32, name="bk")
        nc.scalar.dma_start(bk[:], bk_d[:].rearrange("(o p) -> p o", p=P))
        wvt = _load_w("wvt", wvt_d, CT, eng=nc.scalar)
        bv_b = wts.tile([P, E], FP32, name="bv_b")
        _bcast_row(nc, bv_d, bv_b)

        wq1t = _load_w("wq1t", wq1t_d, CQT)
        bq1 = wts.tile([P, ET], FP32, name="bq1")
        nc.gpsimd.dma_start(bq1[:], bq1_d[:].rearrange("(o p) -> p o", p=P))

        # PE warm-up: ~5us of throwaway matmuls during the DMA ramp flips the
        # HAM clock-gate to 8/8 before the first real projection matmuls.
        wrm = const.tile([P, QB], BF16, name="wrm")
        nc.vector.memset(wrm, 0.0)
        wps = ps_s.tile([P, 2, QB], FP32, name="wps", tag="s")
        NWARM = 8
        for wi in range(NWARM):
            nc.tensor.matmul(
                wps[:, 0, :], wrm[:, :P], wrm[:],
                start=(wi == 0), stop=(wi == NWARM - 1),
            )
        nc.vector.tensor_copy(wrm[:], wps[:, 0, :])
        # preload the exp table set while the PE is still streaming inputs
        nc.scalar.activation(wrm[:, 0:1], wrm[:, 0:1], AF.Exp, scale=1.0)

        QCH = 512

        def _load_xq_raw(xq_d, ch):
            t = bigin.tile([P, CQT, QCH], BF16, name="xq", tag="xq", bufs=3)
            nc.gpsimd.dma_start(
                t[:],
                xq_d[:].rearrange("p (o n) -> p o n", o=CQT)[
                    :, :, ch * QCH : (ch + 1) * QCH
                ],
            )
            return t

        # first q1 chunks land before the remaining (cold-path) weights;
        # chunked loads trickle in during attention without contending with
        # the kv stream the way bulk transfers do
        PREFETCH = 2
        xq_tiles = {i: _load_xq_raw(xq1_d, i) for i in range(PREFETCH)}

        wq2t = _load_w("wq2t", wq2t_d, CQT)
        bq2 = wts.tile([P, ET], FP32, name="bq2")
        nc.gpsimd.dma_start(bq2[:], bq2_d[:].rearrange("(o p) -> p o", p=P))
        wo1t = _load_w("wo1t", wo1t_d, ET)
        wo2t = _load_w("wo2t", wo2t_d, ET)
        bo_b = wts.tile([P, E], FP32, name="bo_b")
        _bcast_row(nc, bo_d, bo_b)
        lnw_b = wts.tile([P, E], FP32, name="lnw_b")
        _bcast_row(nc, lnw_d, lnw_b)
        lnb_b = wts.tile([P, E], FP32, name="lnb_b")
        _bcast_row(nc, lnb_d, lnb_b)

        # ---- constants ----
        ones = const.tile([P, 2], BF16, name="ones")
        nc.vector.memset(ones, 1.0)
        magic = const.tile([P, TPB], I32, name="magic")
        nc.vector.memset(magic, RSQRT_MAGIC)

        # ---- phase 0: K^T, V projections (kv on the sync HWDGE queue) ----
        ktm = keep.tile([P, ET, N], BF16, name="ktm")    # K^T e-major
        vtm = keep.tile([P, NKT, E], BF16, name="vtm")   # V token-major

        qt1 = keep.tile([P, CQT, NQ], BF16, name="qt1")  # Q1^T e-major
        qt2 = keep.tile([P, CQT, NQ], BF16, name="qt2")
        q_specs = [
            (xq_d, wqt, bq, qt, ch)
            for (xq_d, wqt, bq, qt) in (
                (xq1_d, wq1t, bq1, qt1),
                (xq2_d, wq2t, bq2, qt2),
            )
            for ch in range(NQ // QCH)
        ]

        # kv chunks: short ones first for a fast PE start, then long chunks
        # whose contiguous 2KB runs keep the DMA engines off the descriptor
        # bottleneck; alternating HWDGE queues (sync/scalar) double the rate
        KV_CHUNKS = [256, 256, 512, 1024, 1024, 1024]
        kv_off = 0
        for ci, kvch in enumerate(KV_CHUNKS):
            xkv_sb = bigin.tile([P, CT, 1024], BF16, name="xkv", tag="xkv", bufs=3)
            dma_eng = nc.sync if ci % 2 == 0 else nc.scalar
            dma_eng.dma_start(
                xkv_sb[:, :, :kvch],
                xkv_d[:].rearrange("(o p) n -> p o n", p=P)[
                    :, :, kv_off : kv_off + kvch
                ],
            )
            # K^T for these token-columns (ACT drains with per-partition bias)
            for t in range(ET):
                for cc in range(0, kvch, QB):
                    w = min(QB, kvch - cc)
                    ps = ps_o.tile([P, QB], FP32, name="kps", tag="o")
                    for j in range(CT):
                        nc.tensor.matmul(
                            ps[:, :w],
                            wkt[:, j, t * P : (t + 1) * P],
                            xkv_sb[:, j, cc : cc + w],
                            start=(j == 0),
                            stop=(j == CT - 1),
                        )
                    nc.scalar.activation(
                        ktm[:, t, kv_off + cc : kv_off + cc + w],
                        ps[:, :w],
                        AF.Identity,
                        bias=bk[:, t : t + 1],
                        scale=1.0,
                    )
            # V for these token-rows (DVE drains add bv)
            for v in range(kvch // P):
                kt_idx = (kv_off // P) + v
                ps = ps_x.tile([P, E], FP32, name="vps", tag="aux")
                for j in range(CT):
                    nc.tensor.matmul(
                        ps[:],
                        xkv_sb[:, j, v * P : (v + 1) * P],
                        wvt[:, j, :],
                        start=(j == 0),
                        stop=(j == CT - 1),
                    )
                nc.vector.tensor_tensor(vtm[:, kt_idx, :], ps[:], bv_b[:], ALU.add)
            kv_off += kvch

        def _qt_proj_chunk(i):
            """Project one streamed q-input chunk into its Q^T slice."""
            xq_d, wqt, bq, qt, ch = q_specs[i]
            xq_sb = xq_tiles.pop(i)
            nxt = i + PREFETCH
            if nxt < len(q_specs):
                xq_tiles[nxt] = _load_xq_raw(q_specs[nxt][0], q_specs[nxt][4])
            for t in range(ET):
                ps = ps_x.tile([P, QB], FP32, name="qps", tag="aux")
                for j in range(CQT):
                    nc.tensor.matmul(
                        ps[:],
                        wqt[:, j, t * P : (t + 1) * P],
                        xq_sb[:, j, :],
                        start=(j == 0),
                        stop=(j == CQT - 1),
                    )
                nc.scalar.activation(
                    qt[:, t, ch * QCH : (ch + 1) * QCH],
                    ps[:],
                    AF.Identity,
                    bias=bq[:, t : t + 1],
                    scale=1.0,
                )

        # ---- phase 1 + interleaved phase 2 ----
        o1ut = keep.tile([P, ET, NQ], BF16, name="o1ut")  # unnormalized out1^T
        o2ut = keep.tile([P, ET, NQ], BF16, name="o2ut")
        r1 = keep.tile([P, NT], FP32, name="r1")          # 1/denom per token
        r2 = keep.tile([P, NT], FP32, name="r2")

        out_r = out_d[:].rearrange("(nt p) e -> p nt e", p=P)

        def _attn_span(si, qt, out_t, r_t, q_lo, q_w, next_chunk):
            """Attention k-sweep for queries [q_lo, q_lo+q_w)."""
            qsl = slice(q_lo, q_lo + q_w)
            o_ps = [
                ps_o.tile([P, QB], FP32, name=f"ops{t}", tag="o")
                for t in range(ET)
            ]
            acc = flow.tile([P, 2, QB], BF16, name="acc", tag="acc", bufs=2)

            def _emit_pv(pr, pt):
                for half in range(2):
                    k = 2 * pr + half
                    for t in range(ET):
                        nc.tensor.matmul(
                            o_ps[t][:, :q_w],
                            vtm[:, k, t * P : (t + 1) * P],
                            pt[:, half, :q_w],
                            start=(k == 0),
                            stop=(k == NKT - 1),
                        )

            pend = []
            for pr in range(NPAIR):
                # project the NEXT block's q chunk mid-way through this one,
                # so its qt slice is long done before that block starts
                if pr == NPAIR // 2 and next_chunk is not None:
                    _qt_proj_chunk(next_chunk)
                s_ps = ps_s.tile([P, 2, QB], FP32, name="sps", tag="s")
                for half in range(2):
                    k = 2 * pr + half
                    for t in range(ET):
                        nc.tensor.matmul(
                            s_ps[:, half, :q_w],
                            ktm[:, t, k * P : (k + 1) * P],
                            qt[:, t, qsl],
                            start=(t == 0),
                            stop=(t == ET - 1),
                        )
                pt = flow.tile([P, 2, QB], BF16, name="pt", tag="pt", bufs=4)
                nc.scalar.activation(
                    pt[:, :, :q_w], s_ps[:, :, :q_w], AF.Exp, scale=SCALE
                )
                pend.append((pr, pt))
                if len(pend) > PVLAG:
                    _emit_pv(*pend.pop(0))
                if pr == 0:
                    nc.vector.tensor_copy(acc[:, :, :q_w], pt[:, :, :q_w])
                else:
                    nc.vector.tensor_tensor(
                        acc[:, :, :q_w], acc[:, :, :q_w], pt[:, :, :q_w], ALU.add
                    )
            for args in pend:
                _emit_pv(*args)

            for t in range(ET):
                nc.vector.tensor_copy(out_t[:, t, qsl], o_ps[t][:, :q_w])
            # denominators: accumulating ones-matmuls over both acc halves
            d_ps = ps_x.tile([P, TPB, 2], FP32, name="dps", tag="aux")
            nsub = q_w // P
            for i in range(nsub):
                for h in range(2):
                    nc.tensor.matmul(
                        d_ps[:, i, :],
                        acc[:, h, i * P : (i + 1) * P],
                        ones[:],
                        start=(h == 0),
                        stop=(h == 1),
                    )
            nc.vector.reciprocal(
                r_t[:, q_lo // P : q_lo // P + nsub], d_ps[:, :nsub, 0]
            )

        def _phase2a(tiles):
            """Out-proj + softmax-normalize + LayerNorm for given token-tiles."""
            ntl = len(tiles)
            mv = flow.tile([P, TPB, 2], FP32, name="mv", tag="mv", bufs=2)
            ys_list = []
            for i, nt in enumerate(tiles):
                nsl = slice(nt * P, (nt + 1) * P)
                y_ps = ps_x.tile([P, 2, E], FP32, name="yps", tag="aux")
                for h, (out_t, wot) in enumerate(((o1ut, wo1t), (o2ut, wo2t))):
                    for j in range(ET):
                        nc.tensor.matmul(
                            y_ps[:, h, :],
                            out_t[:, j, nsl],
                            wot[:, j, :],
                            start=(j == 0),
                            stop=(j == ET - 1),
                        )
                yb = flow.tile([P, 2, E], FP32, name="yb", tag="yb", bufs=2)
                for h, r_t in enumerate((r1, r2)):
                    nc.scalar.activation(
                        yb[:, h, :], y_ps[:, h, :], AF.Identity,
                        scale=r_t[:, nt : nt + 1],
                    )
                ys = flow.tile([P, E], FP32, name="ys", tag="ys", bufs=2 * TPB)
                nc.vector.tensor_tensor(ys[:], yb[:, 0, :], yb[:, 1, :], ALU.add)
                nc.vector.tensor_tensor(ys[:], ys[:], bo_b[:], ALU.add)
                st6 = flow.tile([P, 6], FP32, name="st6", tag="st6", bufs=2)
                nc.vector.bn_stats(out=st6[:], in_=ys[:])
                nc.vector.bn_aggr(out=mv[:, i, :], in_=st6[:])
                ys_list.append(ys)
            # rstd = 1/sqrt(var+eps) on DVE (magic rsqrt + 2 Newton steps)
            rs = flow.tile([P, TPB], FP32, name="rs", tag="rs", bufs=2)
            t4 = flow.tile([P, TPB], FP32, name="t4", tag="t4", bufs=2)
            x4 = flow.tile([P, TPB], FP32, name="x4", tag="x4", bufs=2)
            nc.vector.tensor_scalar(
                x4[:, :ntl], mv[:, :ntl, 1], LN_EPS, None, op0=ALU.add
            )
            nc.vector.tensor_scalar(
                rs[:, :ntl].bitcast(I32), x4[:, :ntl].bitcast(I32), 1, None,
                op0=ALU.logical_shift_right,
            )
            nc.vector.tensor_tensor(
                rs[:, :ntl].bitcast(I32), magic[:, :ntl],
                rs[:, :ntl].bitcast(I32), ALU.subtract,
            )
            # one Newton step (max rel err ~1.8e-3 on rstd; well inside budget)
            for _ in range(1):
                nc.vector.tensor_tensor(t4[:, :ntl], x4[:, :ntl], rs[:, :ntl], ALU.mult)
                nc.vector.tensor_tensor(t4[:, :ntl], t4[:, :ntl], rs[:, :ntl], ALU.mult)
                nc.vector.tensor_scalar(
                    t4[:, :ntl], t4[:, :ntl], -0.5, 1.5, op0=ALU.mult, op1=ALU.add
                )
                nc.vector.tensor_tensor(rs[:, :ntl], rs[:, :ntl], t4[:, :ntl], ALU.mult)
            # normalize + affine, then store token-major (host transposes)
            for i, nt in enumerate(tiles):
                ys = ys_list[i]
                yf = flow.tile([P, E], FP32, name="yf", tag="yf", bufs=2 * TPB)
                nc.vector.tensor_scalar(
                    yf[:], ys[:], mv[:, i, 0:1], rs[:, i : i + 1],
                    op0=ALU.subtract, op1=ALU.mult,
                )
                nc.vector.tensor_tensor(yf[:], yf[:], lnw_b[:], ALU.mult)
                nc.vector.tensor_tensor(yf[:], yf[:], lnb_b[:], ALU.add)
                nc.sync.dma_start(out_r[:, nt, :], yf[:])

        _qt_proj_chunk(0)
        for qb in range(NQB):                      # set 1 (q1): attention only
            _attn_span(0, qt1, o1ut, r1, qb * QB, QB, qb + 1)
        for qb in range(NQB - 1):                  # set 2 (q2): attn + phase 2
            nxt = NQB + qb + 1 if NQB + qb + 1 < len(q_specs) else None
            _attn_span(1, qt2, o2ut, r2, qb * QB, QB, nxt)
            _phase2a([qb * TPB + i for i in range(TPB)])
        # final block split into two query halves so only ~2 token-tiles of
        # LN/store work remain exposed after the last matmul
        HB = QB // 2
        q0 = (NQB - 1) * QB
        _attn_span(1, qt2, o2ut, r2, q0, HB, None)
        _phase2a([q0 // P, q0 // P + 1])
        _attn_span(1, qt2, o2ut, r2, q0 + HB, HB, None)
        _phase2a([(q0 + HB) // P, (q0 + HB) // P + 1])

    nc.compile()
    return nc


_CACHE = {}


def _get_nc():
    if "nc" not in _CACHE:
        _CACHE["nc"] = build_nc()
    return _CACHE["nc"]


def make_in_maps(q1, q2, kv, wq1, bq1, wq2, bq2, wk, bk, wv, bv, wo, bo, ln_w, ln_b):
    bf = lambda a: np.ascontiguousarray(
        np.asarray(a, dtype=np.float32).astype(ml_dtypes.bfloat16)
    )
    f32 = lambda a: np.ascontiguousarray(np.asarray(a, dtype=np.float32))

    def sharded(wt):
        # [C, E] -> [P, (C//P)*E] in the on-chip [p][o][e] layout
        c, e = wt.shape
        return bf(wt.reshape(c // P, P, e).transpose(1, 0, 2).reshape(P, -1))

    q1, q2, kv = np.asarray(q1), np.asarray(q2), np.asarray(kv)
    base = {
        "wq1t": sharded(np.asarray(wq1).T),
        "wq2t": sharded(np.asarray(wq2).T),
        "wkt": sharded(np.asarray(wk).T),
        "wvt": sharded(np.asarray(wv).T),
        "wo1t": sharded(np.asarray(wo)[:, :E].T),
        "wo2t": sharded(np.asarray(wo)[:, E:].T),
        "bq1": f32(bq1),
        "bq2": f32(bq2),
        "bk": f32(bk),
        "bv": f32(bv),
        "bo": f32(bo),
        "lnw": f32(ln_w),
        "lnb": f32(ln_b),
    }
    kv_flat = [bf(kv[b].reshape(CKV, N)) for b in range(B)]
    in_maps = []
    for c in range(8):
        b, h = divmod(c, 2)
        m = dict(base)
        m["xq1"] = sharded(q1[b, :, h * 32 : (h + 1) * 32, :].reshape(CQ, NQ))
        m["xq2"] = sharded(q2[b, :, h * 32 : (h + 1) * 32, :].reshape(CQ, NQ))
        m["xkv"] = kv_flat[b]
        in_maps.append(m)
    return in_maps


def assemble_output(results):
    out = np.empty((B, E, 64, 64), dtype=np.float32)
    for c in range(8):
        b, h = divmod(c, 2)
        y = np.asarray(results[c]["out"], dtype=np.float32)  # [NQ, E]
        out[b, :, h * 32 : (h + 1) * 32, :] = y.T.reshape(E, 32, 64)
    return out


def kernel(**inputs):
    from concourse.bass_utils import run_bass_kernel_spmd

    nc = _get_nc()
    in_maps = make_in_maps(**inputs)
    res = run_bass_kernel_spmd(nc, in_maps, list(range(8)))
    return assemble_output(res.results)


if __name__ == "__main__":
    nc = build_nc()
    print("built ok")
